# revision 2
# baseline (speedup 1.0000x reference)
"""Trainium2 Bass kernel for nn_ConvExponential: kg = sum_{i=0..5} K^(*i)/i!
where K^(*i) is the i-fold conv-composition of the 3x3 kernel with itself.

Strategy: shard the in-channel axis (n, axis 1) of the running power across
8 cores (64 channels each). Each core iterates ki <- (K (*) ki)/i locally as
a sequence of TensorE matmuls over channel chunks: for each spatial tap t and
output row, R[b, n, q] += K[b, m, t] @ ki[m, n, q - t]. lhsT operands are the
9 taps of K^T ([m, b] layout), rhs is the ki slice laid out [m, (px, py, n)]
so a whole spatial row batches into one matmul (free dim = W_in*64).
Matmuls run in fp16 (1 cycle/row on TRN2 PE) accumulating in fp32 PSUM.
Each iteration's scaled term is evacuated (DVE, *1/i, cast fp16) and DMAd to
DRAM; the same SBUF tiles feed the next iteration. The host adds the exact
i=0,1 terms (identity + K) in fp32 and assembles the final (512,512,11,11).
"""

import sys

if "/opt/trn_rl_repo" not in sys.path:
    sys.path.insert(0, "/opt/trn_rl_repo")

import numpy as np

NC = 512      # channels
S = 8         # cores
NS = NC // S  # 64 per-core slice
P = 128       # partitions
MC = NC // P  # 4 contraction chunks

_cache = {}


def _build():
    import concourse.bacc as bacc
    import concourse.mybir as mybir
    import concourse.tile as tile

    f16 = mybir.dt.float16
    f32 = mybir.dt.float32

    nc = bacc.Bacc("TRN2", target_bir_lowering=False, debug=False, num_devices=S)
    kt_d = nc.dram_tensor("kt", [MC, P, 9 * NC], f16, kind="ExternalInput")
    ki1_d = nc.dram_tensor("ki1", [MC, P, 9 * NS], f16, kind="ExternalInput")
    out_d = {}
    for it in range(2, 6):
        W = 2 * it + 1
        out_d[it] = nc.dram_tensor(
            f"out{it}", [MC, P, W * W * NS], f16, kind="ExternalOutput"
        )

    with tile.TileContext(nc) as tc:
        with (
            tc.tile_pool(name="ktp", bufs=1) as kt_pool,
            tc.tile_pool(name="kip", bufs=1) as ki_pool,
            tc.tile_pool(name="stp", bufs=8) as st_pool,
            tc.tile_pool(name="psp", bufs=8, space="PSUM") as ps_pool,
        ):
            kts = []
            for mc in range(MC):
                t = kt_pool.tile([P, 9 * NC], f16, tag=f"kt{mc}", name=f"kt{mc}")
                nc.sync.dma_start(t[:], kt_d[mc])
                kts.append(t)
            ki_cur = []
            for mc in range(MC):
                t = ki_pool.tile([P, 9 * NS], f16, tag=f"ki1_{mc}", name=f"ki1t_{mc}")
                nc.sync.dma_start(t[:], ki1_d[mc])
                ki_cur.append(t)

            for it in range(2, 6):
                w_in, w_out = 2 * it - 1, 2 * it + 1
                scale = 1.0 / it
                if w_out * NS <= 512:
                    chunks = [(0, w_out)]
                else:
                    mid = (w_out + 1) // 2
                    chunks = [(0, mid), (mid, w_out)]
                ki_new = None
                if it < 5:
                    ki_new = [
                        ki_pool.tile([P, w_out * w_out * NS], f16, tag=f"ki{it}_{mc}", name=f"ki{it}_{mc}")
                        for mc in range(MC)
                    ]
                for qx in range(w_out):
                    for bc in range(MC):
                        for (y0, y1) in chunks:
                            ps = ps_pool.tile([P, 512], f32, tag="ps", name="ps")
                            mms = []
                            for t1 in range(3):
                                px = qx - t1
                                if not (0 <= px < w_in):
                                    continue
                                for t2 in range(3):
                                    qa = max(y0, t2)
                                    qb = min(y1, t2 + w_in)
                                    if qa >= qb:
                                        continue
                                    for mc in range(MC):
                                        mms.append((t1, t2, mc, px, qa, qb))
                            n = len(mms)
                            for idx, (t1, t2, mc, px, qa, qb) in enumerate(mms):
                                tt = t1 * 3 + t2
                                lhsT = kts[mc][:, tt * NC + bc * P : tt * NC + bc * P + P]
                                rhs = ki_cur[mc][
                                    :, (px * w_in + qa - t2) * NS : (px * w_in + qb - t2) * NS
                                ]
                                nc.tensor.matmul(
                                    ps[:, (qa - y0) * NS : (qb - y0) * NS],
                                    lhsT,
                                    rhs,
                                    start=(idx == 0),
                                    stop=(idx == n - 1),
                                )
                            wlen = (y1 - y0) * NS
                            col0 = (qx * w_out + y0) * NS
                            col1 = (qx * w_out + y1) * NS
                            if it < 5:
                                dst = ki_new[bc][:, col0:col1]
                                nc.vector.tensor_scalar_mul(dst, ps[:, :wlen], scale)
                                nc.sync.dma_start(out_d[it][bc, :, col0:col1], dst)
                            else:
                                st = st_pool.tile([P, 512], f16, tag="st", name="st")
                                nc.vector.tensor_scalar_mul(st[:, :wlen], ps[:, :wlen], scale)
                                nc.sync.dma_start(
                                    out_d[it][bc, :, col0:col1], st[:, :wlen]
                                )
                if it < 5:
                    ki_cur = ki_new
    nc.compile()
    return nc


def _run(kern, trace=False):
    """kern: (512, 512, 3, 3) float32. Returns (results_list, exec_time_ns)."""
    from concourse.bass_utils import run_bass_kernel_spmd

    if "nc" not in _cache:
        _cache["nc"] = _build()
    nc = _cache["nc"]

    # lhsT taps: [m, t1, t2, b] -> (MC, P, 9*NC)
    kt_host = (
        np.transpose(kern, (1, 2, 3, 0))
        .reshape(MC, P, 9 * NC)
        .astype(np.float16)
    )
    in_maps = []
    for c in range(S):
        sl = kern[:, c * NS : (c + 1) * NS]  # (512, NS, 3, 3)
        ki1 = (
            np.transpose(sl, (0, 2, 3, 1)).reshape(MC, P, 9 * NS).astype(np.float16)
        )
        in_maps.append({"kt": kt_host, "ki1": ki1})
    r = run_bass_kernel_spmd(nc, in_maps, list(range(S)), trace=trace)
    return r.results, r.exec_time_ns


def _assemble(kern, results):
    kg = np.zeros((NC, NC, 11, 11), np.float32)
    kg[:, :, 4:7, 4:7] += kern
    kg[np.arange(NC), np.arange(NC), 5, 5] += 1.0
    for c in range(S):
        for it in range(2, 6):
            W = 2 * it + 1
            off = 5 - it
            a = results[c][f"out{it}"].astype(np.float32).reshape(NC, W, W, NS)
            kg[:, c * NS : (c + 1) * NS, off : off + W, off : off + W] += np.transpose(
                a, (0, 3, 1, 2)
            )
    return kg


def kernel(**inputs):
    kern = np.asarray(inputs["kernel"], dtype=np.float32)
    results, _ = _run(kern, trace=False)
    return _assemble(kern, results)


# revision 3
# speedup vs baseline: 1.0048x; 1.0048x over previous
"""Trainium2 Bass kernel for nn_ConvExponential: kg = sum_{i=0..5} K^(*i)/i!
where K^(*i) is the i-fold conv-composition of the 3x3 kernel with itself.

Strategy: shard the in-channel axis (n, axis 1) of the running power across
8 cores (64 channels each). Each core iterates ki <- (K (*) ki)/i locally as
a sequence of TensorE matmuls over channel chunks: for each spatial tap t and
output row, R[b, n, q] += K[b, m, t] @ ki[m, n, q - t]. lhsT operands are the
9 taps of K^T ([m, b] layout), rhs is the ki slice laid out [m, (px, py, n)]
so a whole spatial row batches into one matmul (free dim = W_in*64).
Matmuls run in fp16 (1 cycle/row on TRN2 PE) accumulating in fp32 PSUM.
Each iteration's scaled term is evacuated (DVE, *1/i, cast fp16) and DMAd to
DRAM; the same SBUF tiles feed the next iteration. The host adds the exact
i=0,1 terms (identity + K) in fp32 and assembles the final (512,512,11,11).
"""

import sys

if "/opt/trn_rl_repo" not in sys.path:
    sys.path.insert(0, "/opt/trn_rl_repo")

import numpy as np

NC = 512      # channels
S = 8         # cores
NS = NC // S  # 64 per-core slice
P = 128       # partitions
MC = NC // P  # 4 contraction chunks

_cache = {}


def _build():
    import concourse.bacc as bacc
    import concourse.mybir as mybir
    import concourse.tile as tile

    f16 = mybir.dt.float16
    f32 = mybir.dt.float32

    nc = bacc.Bacc("TRN2", target_bir_lowering=False, debug=False, num_devices=S)
    kt_d = nc.dram_tensor("kt", [MC, P, 9 * NC], f16, kind="ExternalInput")
    ki1_d = nc.dram_tensor("ki1", [MC, P, 9 * NS], f16, kind="ExternalInput")
    out_d = {}
    for it in range(2, 6):
        W = 2 * it + 1
        out_d[it] = nc.dram_tensor(
            f"out{it}", [MC, P, W * W * NS], f16, kind="ExternalOutput"
        )

    with tile.TileContext(nc) as tc:
        with (
            tc.tile_pool(name="ktp", bufs=1) as kt_pool,
            tc.tile_pool(name="kip", bufs=1) as ki_pool,
            tc.tile_pool(name="stp", bufs=8) as st_pool,
            tc.tile_pool(name="psp", bufs=8, space="PSUM") as ps_pool,
        ):
            ki_cur = []
            for mc in range(MC):
                t = ki_pool.tile([P, 9 * NS], f16, tag=f"ki1_{mc}", name=f"ki1t_{mc}")
                nc.sync.dma_start(t[:], ki1_d[mc])
                ki_cur.append(t)
            # per-tap weight loads, in first-use order, so the first matmul
            # only waits for tap 0 instead of the whole 4.7MB weight tensor
            kts = [
                kt_pool.tile([P, 9 * NC], f16, tag=f"kt{mc}", name=f"kt{mc}")
                for mc in range(MC)
            ]
            for tt in range(9):
                for mc in range(MC):
                    nc.sync.dma_start(
                        kts[mc][:, tt * NC : (tt + 1) * NC],
                        kt_d[mc, :, tt * NC : (tt + 1) * NC],
                    )

            for it in range(2, 6):
                w_in, w_out = 2 * it - 1, 2 * it + 1
                scale = 1.0 / it
                if w_out * NS <= 512:
                    chunks = [(0, w_out)]
                else:
                    mid = (w_out + 1) // 2
                    chunks = [(0, mid), (mid, w_out)]
                ki_new = None
                if it < 5:
                    ki_new = [
                        ki_pool.tile([P, w_out * w_out * NS], f16, tag=f"ki{it}_{mc}", name=f"ki{it}_{mc}")
                        for mc in range(MC)
                    ]
                for qx in range(w_out):
                    for bc in range(MC):
                        for (y0, y1) in chunks:
                            ps = ps_pool.tile([P, 512], f32, tag="ps", name="ps")
                            mms = []
                            for t1 in range(3):
                                px = qx - t1
                                if not (0 <= px < w_in):
                                    continue
                                for t2 in range(3):
                                    qa = max(y0, t2)
                                    qb = min(y1, t2 + w_in)
                                    if qa >= qb:
                                        continue
                                    for mc in range(MC):
                                        mms.append((t1, t2, mc, px, qa, qb))
                            n = len(mms)
                            for idx, (t1, t2, mc, px, qa, qb) in enumerate(mms):
                                tt = t1 * 3 + t2
                                lhsT = kts[mc][:, tt * NC + bc * P : tt * NC + bc * P + P]
                                rhs = ki_cur[mc][
                                    :, (px * w_in + qa - t2) * NS : (px * w_in + qb - t2) * NS
                                ]
                                nc.tensor.matmul(
                                    ps[:, (qa - y0) * NS : (qb - y0) * NS],
                                    lhsT,
                                    rhs,
                                    start=(idx == 0),
                                    stop=(idx == n - 1),
                                )
                            wlen = (y1 - y0) * NS
                            col0 = (qx * w_out + y0) * NS
                            col1 = (qx * w_out + y1) * NS
                            if it < 5:
                                dst = ki_new[bc][:, col0:col1]
                                nc.vector.tensor_scalar_mul(dst, ps[:, :wlen], scale)
                                nc.sync.dma_start(out_d[it][bc, :, col0:col1], dst)
                            else:
                                st = st_pool.tile([P, 512], f16, tag="st", name="st")
                                nc.vector.tensor_scalar_mul(st[:, :wlen], ps[:, :wlen], scale)
                                nc.sync.dma_start(
                                    out_d[it][bc, :, col0:col1], st[:, :wlen]
                                )
                if it < 5:
                    ki_cur = ki_new
    nc.compile()
    return nc


def _run(kern, trace=False):
    """kern: (512, 512, 3, 3) float32. Returns (results_list, exec_time_ns)."""
    from concourse.bass_utils import run_bass_kernel_spmd

    if "nc" not in _cache:
        _cache["nc"] = _build()
    nc = _cache["nc"]

    # lhsT taps: [m, t1, t2, b] -> (MC, P, 9*NC)
    kt_host = (
        np.transpose(kern, (1, 2, 3, 0))
        .reshape(MC, P, 9 * NC)
        .astype(np.float16)
    )
    in_maps = []
    for c in range(S):
        sl = kern[:, c * NS : (c + 1) * NS]  # (512, NS, 3, 3)
        ki1 = (
            np.transpose(sl, (0, 2, 3, 1)).reshape(MC, P, 9 * NS).astype(np.float16)
        )
        in_maps.append({"kt": kt_host, "ki1": ki1})
    r = run_bass_kernel_spmd(nc, in_maps, list(range(S)), trace=trace)
    return r.results, r.exec_time_ns


def _assemble(kern, results):
    kg = np.zeros((NC, NC, 11, 11), np.float32)
    kg[:, :, 4:7, 4:7] += kern
    kg[np.arange(NC), np.arange(NC), 5, 5] += 1.0
    for c in range(S):
        for it in range(2, 6):
            W = 2 * it + 1
            off = 5 - it
            a = results[c][f"out{it}"].astype(np.float32).reshape(NC, W, W, NS)
            kg[:, c * NS : (c + 1) * NS, off : off + W, off : off + W] += np.transpose(
                a, (0, 3, 1, 2)
            )
    return kg


def kernel(**inputs):
    kern = np.asarray(inputs["kernel"], dtype=np.float32)
    results, _ = _run(kern, trace=False)
    return _assemble(kern, results)


# revision 5
# speedup vs baseline: 1.0060x; 1.0011x over previous
"""Trainium2 Bass kernel for nn_ConvExponential: kg = sum_{i=0..5} K^(*i)/i!
where K^(*i) is the i-fold conv-composition of the 3x3 kernel with itself.

Strategy: shard the in-channel axis (n, axis 1) of the running power across
8 cores (64 channels each). Each core iterates ki <- (K (*) ki)/i locally as
a sequence of TensorE matmuls over channel chunks: for each spatial tap t and
output row, R[b, n, q] += K[b, m, t] @ ki[m, n, q - t]. lhsT operands are the
9 taps of K^T ([m, b] layout), rhs is the ki slice laid out [m, (px, py, n)]
so a whole spatial row batches into one matmul (free dim = W_in*64).
Matmuls run in fp16 (1 cycle/row on TRN2 PE) accumulating in fp32 PSUM.
Each iteration's scaled term is evacuated (DVE, *1/i, cast fp16) and DMAd to
DRAM; the same SBUF tiles feed the next iteration. The host adds the exact
i=0,1 terms (identity + K) in fp32 and assembles the final (512,512,11,11).
"""

import sys

if "/opt/trn_rl_repo" not in sys.path:
    sys.path.insert(0, "/opt/trn_rl_repo")

import numpy as np

NC = 512      # channels
S = 8         # cores
NS = NC // S  # 64 per-core slice
P = 128       # partitions
MC = NC // P  # 4 contraction chunks

_cache = {}


def _build():
    import concourse.bacc as bacc
    import concourse.mybir as mybir
    import concourse.tile as tile

    f16 = mybir.dt.float16
    f32 = mybir.dt.float32

    nc = bacc.Bacc("TRN2", target_bir_lowering=False, debug=False, num_devices=S)
    kt_d = nc.dram_tensor("kt", [MC, P, 9 * NC], f16, kind="ExternalInput")
    ki1_d = nc.dram_tensor("ki1", [MC, P, 9 * NS], f16, kind="ExternalInput")
    out_d = {}
    for it in range(2, 6):
        W = 2 * it + 1
        out_d[it] = nc.dram_tensor(
            f"out{it}", [MC, P, W * W * NS], f16, kind="ExternalOutput"
        )

    with tile.TileContext(nc) as tc:
        with (
            tc.tile_pool(name="ktp", bufs=1) as kt_pool,
            tc.tile_pool(name="kip", bufs=1) as ki_pool,
            tc.tile_pool(name="stp", bufs=8) as st_pool,
            tc.tile_pool(name="psp", bufs=8, space="PSUM") as ps_pool,
        ):
            # PE prewarm: dummy matmuls on a zeroed scratch tile while the
            # input DMAs land, so the HAM clock-gate reaches 8/8 before the
            # real stream begins and the PE isn't idle during the load.
            warm = kt_pool.tile([P, 512], f16, tag="warm", name="warm")
            warm_ps = ps_pool.tile([P, 512], f32, tag="ps", name="warmps")
            nc.vector.memset(warm[:], 0.0)
            for _ in range(15):
                nc.tensor.matmul(
                    warm_ps[:], warm[:, 0:P], warm[:], start=True, stop=True
                )

            ki_cur = []
            for mc in range(MC):
                t = ki_pool.tile([P, 9 * NS], f16, tag=f"ki1_{mc}", name=f"ki1t_{mc}")
                nc.sync.dma_start(t[:], ki1_d[mc])
                ki_cur.append(t)
            # per-tap weight loads, in first-use order, so the first matmul
            # only waits for tap 0 instead of the whole 4.7MB weight tensor
            kts = [
                kt_pool.tile([P, 9 * NC], f16, tag=f"kt{mc}", name=f"kt{mc}")
                for mc in range(MC)
            ]
            for tt in range(9):
                for mc in range(MC):
                    nc.sync.dma_start(
                        kts[mc][:, tt * NC : (tt + 1) * NC],
                        kt_d[mc, :, tt * NC : (tt + 1) * NC],
                    )

            for it in range(2, 6):
                w_in, w_out = 2 * it - 1, 2 * it + 1
                scale = 1.0 / it
                if w_out * NS <= 512:
                    chunks = [(0, w_out)]
                else:
                    mid = (w_out + 1) // 2
                    chunks = [(0, mid), (mid, w_out)]
                ki_new = None
                if it < 5:
                    ki_new = [
                        ki_pool.tile([P, w_out * w_out * NS], f16, tag=f"ki{it}_{mc}", name=f"ki{it}_{mc}")
                        for mc in range(MC)
                    ]
                for qx in range(w_out):
                    for bc in range(MC):
                        for (y0, y1) in chunks:
                            ps = ps_pool.tile([P, 512], f32, tag="ps", name="ps")
                            mms = []
                            for t1 in range(3):
                                px = qx - t1
                                if not (0 <= px < w_in):
                                    continue
                                for t2 in range(3):
                                    qa = max(y0, t2)
                                    qb = min(y1, t2 + w_in)
                                    if qa >= qb:
                                        continue
                                    for mc in range(MC):
                                        mms.append((t1, t2, mc, px, qa, qb))
                            n = len(mms)
                            for idx, (t1, t2, mc, px, qa, qb) in enumerate(mms):
                                tt = t1 * 3 + t2
                                lhsT = kts[mc][:, tt * NC + bc * P : tt * NC + bc * P + P]
                                rhs = ki_cur[mc][
                                    :, (px * w_in + qa - t2) * NS : (px * w_in + qb - t2) * NS
                                ]
                                nc.tensor.matmul(
                                    ps[:, (qa - y0) * NS : (qb - y0) * NS],
                                    lhsT,
                                    rhs,
                                    start=(idx == 0),
                                    stop=(idx == n - 1),
                                )
                            wlen = (y1 - y0) * NS
                            col0 = (qx * w_out + y0) * NS
                            col1 = (qx * w_out + y1) * NS
                            if it < 5:
                                dst = ki_new[bc][:, col0:col1]
                                nc.vector.tensor_scalar_mul(dst, ps[:, :wlen], scale)
                                nc.sync.dma_start(out_d[it][bc, :, col0:col1], dst)
                            else:
                                st = st_pool.tile([P, 512], f16, tag="st", name="st")
                                nc.vector.tensor_scalar_mul(st[:, :wlen], ps[:, :wlen], scale)
                                nc.sync.dma_start(
                                    out_d[it][bc, :, col0:col1], st[:, :wlen]
                                )
                if it < 5:
                    ki_cur = ki_new
    nc.compile()
    return nc


def _run(kern, trace=False):
    """kern: (512, 512, 3, 3) float32. Returns (results_list, exec_time_ns)."""
    from concourse.bass_utils import run_bass_kernel_spmd

    if "nc" not in _cache:
        _cache["nc"] = _build()
    nc = _cache["nc"]

    # lhsT taps: [m, t1, t2, b] -> (MC, P, 9*NC)
    kt_host = (
        np.transpose(kern, (1, 2, 3, 0))
        .reshape(MC, P, 9 * NC)
        .astype(np.float16)
    )
    in_maps = []
    for c in range(S):
        sl = kern[:, c * NS : (c + 1) * NS]  # (512, NS, 3, 3)
        ki1 = (
            np.transpose(sl, (0, 2, 3, 1)).reshape(MC, P, 9 * NS).astype(np.float16)
        )
        in_maps.append({"kt": kt_host, "ki1": ki1})
    r = run_bass_kernel_spmd(nc, in_maps, list(range(S)), trace=trace)
    return r.results, r.exec_time_ns


def _assemble(kern, results):
    kg = np.zeros((NC, NC, 11, 11), np.float32)
    kg[:, :, 4:7, 4:7] += kern
    kg[np.arange(NC), np.arange(NC), 5, 5] += 1.0
    for c in range(S):
        for it in range(2, 6):
            W = 2 * it + 1
            off = 5 - it
            a = results[c][f"out{it}"].astype(np.float32).reshape(NC, W, W, NS)
            kg[:, c * NS : (c + 1) * NS, off : off + W, off : off + W] += np.transpose(
                a, (0, 3, 1, 2)
            )
    return kg


def kernel(**inputs):
    kern = np.asarray(inputs["kernel"], dtype=np.float32)
    results, _ = _run(kern, trace=False)
    return _assemble(kern, results)
